# revision 17
# baseline (speedup 1.0000x reference)
"""Trainium2 Bass kernel for nn_Encoder_57380763074770.

GRU-cell encoder over 64 independent "steps":
  xi  = concat(x[64,17], ip_emb[ip].reshape(64,8), port_emb[port].reshape(64,8))
  G   = xi @ W_ih.T + h0 @ W_hh.T + (b_ih + b_hh)       # [64, 384]
  r, z = sigmoid(G_r), sigmoid(G_z)
  n   = tanh(G_n + (r - 1) * hn),  hn = h0 @ W_hh_n.T + b_hh_n
  out = n + z * (h0 - n)                                # [64, 128]

Sharding: H=128 hidden columns split 8 ways -> each core owns 16 columns of
every gate (48 rows of W_ih/W_hh) and computes out[:, 16c:16c+16].

Layout decisions (driven by the HW profile -- fixed costs dominate at this
size: ~0.6us per dma_start issue, ~1.5-2us DMA completion latency, ~1.1us
per indirect DMA on the Q7, ~1.3us per activation-table load):
- Weights/x ride HOST-TRANSPOSED (contraction-major) inside ONE packed
  [128, 625] f32 DMA, so the PE does no weight transposes at all.
- Port indices and ip indices ride in one tiny [128, 5] i32 DMA that lands
  first; the replicated 256-entry ip table rides on the Scalar-engine HWDGE
  in parallel.
- The 512 ip_emb lookups are computed on the VECTOR engine (128 partitions,
  partition j*64+s holds columns k=4j+g): int32 iota (generated on GpSimd)
  + one-hot is_equal + multiply + blocked 3D reduce.
- The port gather (70000x4 table) is ONE indirect DMA of 128 row lookups.
- G is accumulated in PSUM h-parts/bias/x first; only the final two
  matmuls wait on the DVE ip path.
- Activation tables pre-warmed against a memset scratch so both loads
  overlap the input DMAs.
"""

import numpy as np

import concourse.bacc as bacc
import concourse.bass as bass
import concourse.mybir as mybir
import concourse.tile as tile
from concourse.bass_utils import run_bass_kernel_spmd

STEPS = 64
H = 128
N_CORES = 8
HS = H // N_CORES       # hidden cols per core = 16
G3 = 3 * HS             # gate rows per core = 48

F32 = mybir.dt.float32
I32 = mybir.dt.int32

# packed params column layout ("pk", [128, F_PK]); all W blocks contraction-major
C_WHT = 0               # [0:128, 0:48]    W_hh slice, transposed
C_WX = 48               # [0:17, 48:96]    W_ih x-features, transposed
C_WIP0 = 96             # [0:4, 96:144]    W_ih ip k=0..3, transposed
C_WIP1 = 144            # [0:4, 144:192]   W_ih ip k=4..7, transposed
C_WPT0 = 192            # [0:4, 192:240]   W_ih port k=0, transposed
C_WPT1 = 240            # [0:4, 240:288]   W_ih port k=1, transposed
C_XT = 288              # [0:17, 288:352]  x transposed
C_ID = 352              # [0:128, 352:480] identity
C_H = 480               # [0:128, 480]     h0 column
C_B = 481               # [0, 481:529]     b_ih + b_hh slice
C_BN = 529              # [0, 529:545]     b_hh n-gate slice
C_H0 = 545              # [0, 545:561]     h0 slice for this core
C_ONE = 561             # [0, 561:625]     ones row
F_PK = 625

V = 256                 # ip table size


def build_nc():
    nc = bacc.Bacc(None)

    idx_d = nc.declare_dram_parameter("idx", [H, 1], I32, isOutput=False)
    dv_d = nc.declare_dram_parameter("dv", [H, 2 * V + 4], F32, isOutput=False)
    pk_d = nc.declare_dram_parameter("pk", [H, F_PK], F32, isOutput=False)
    pte_d = nc.declare_dram_parameter("port_emb", [70000, 4], F32, isOutput=False)
    out_d = nc.declare_dram_parameter("out", [STEPS, HS], F32, isOutput=True)

    with tile.TileContext(nc) as tc:
        with (
            tc.tile_pool(name="sb", bufs=1) as sb,
            tc.tile_pool(name="ps", bufs=1, space="PSUM") as ps,
        ):
            idx = sb.tile([H, 1], I32)
            dv = sb.tile([H, 2 * V + 4], F32)
            pk = sb.tile([H, F_PK], F32)
            st = sb.tile([H, 4], F32)
            wsrc = sb.tile([1, 1], F32)
            warm = sb.tile([1, 2], F32)
            pr = sb.tile([H, V], F32)
            ipf = sb.tile([H, 4], F32)
            ipT = sb.tile([4, H], F32)
            ptT = sb.tile([4, H], F32)
            rz = sb.tile([STEPS, 2 * HS], F32)
            t2 = sb.tile([STEPS, HS], F32)
            u = sb.tile([STEPS, HS], F32)
            n = sb.tile([STEPS, HS], F32)
            zz = sb.tile([STEPS, HS], F32)
            zh = sb.tile([STEPS, HS], F32)
            m = sb.tile([STEPS, HS], F32)
            o = sb.tile([STEPS, HS], F32)

            # DMAs all on the Sync HWDGE queue: port idx, DVE block, params
            nc.sync.dma_start(out=idx[:], in_=idx_d[:, :])
            nc.sync.dma_start(out=dv[:], in_=dv_d[:, :])
            nc.sync.dma_start(out=pk[:], in_=pk_d[:, :])

            # gpsimd: scratch memset + ONLY the port gather (clean Q7 state)
            nc.gpsimd.memset(wsrc[:], 0.25)
            nc.gpsimd.indirect_dma_start(
                out=st[:],
                out_offset=None,
                in_=pte_d[:, :],
                in_offset=bass.IndirectOffsetOnAxis(ap=idx[:, 0:1], axis=0),
            )

            # pre-warm both activation tables (reads only the memset scratch)
            nc.scalar.activation(warm[:, 0:1], wsrc[:],
                                 mybir.ActivationFunctionType.Tanh)
            nc.scalar.activation(warm[:, 1:2], wsrc[:],
                                 mybir.ActivationFunctionType.Sigmoid)

            # ip embedding on DVE (partition j*64+s holds k = 4j+g): per column
            # ONE fused op: (iota == idx_k) * table, accumulated to a scalar
            A = mybir.AluOpType
            iof = dv[:, V : 2 * V]
            emb = dv[:, 0:V]
            for k in range(4):
                nc.vector.scalar_tensor_tensor(
                    out=pr[:, :], in0=iof, scalar=dv[:, 2 * V + k : 2 * V + k + 1],
                    in1=emb, op0=A.is_equal, op1=A.mult,
                    accum_out=ipf[:, k : k + 1])

            ident = pk[:, C_ID : C_ID + H]
            hcol_b = pk[:, C_H : C_H + 1].to_broadcast([H, STEPS])
            ones = pk[0:1, C_ONE : C_ONE + STEPS]

            # h-dependent matmuls (all inputs direct from pk)
            HN = ps.tile([STEPS, HS], F32, space="PSUM")
            nc.tensor.matmul(out=HN[:], lhsT=hcol_b,
                             rhs=pk[0:H, C_WHT + 2 * HS : C_WHT + 3 * HS],
                             start=True, stop=False)
            nc.tensor.matmul(out=HN[:], lhsT=ones, rhs=pk[0:1, C_BN : C_BN + HS],
                             start=False, stop=True)

            H0B = ps.tile([STEPS, HS], F32, space="PSUM")
            nc.tensor.matmul(out=H0B[:], lhsT=ones, rhs=pk[0:1, C_H0 : C_H0 + HS],
                             start=True, stop=True)

            # G accumulation: everything not gather/ip-dependent first
            G = ps.tile([STEPS, G3], F32, space="PSUM")
            nc.tensor.matmul(out=G[:], lhsT=hcol_b, rhs=pk[0:H, C_WHT : C_WHT + G3],
                             start=True, stop=False)
            nc.tensor.matmul(out=G[:], lhsT=ones, rhs=pk[0:1, C_B : C_B + G3],
                             start=False, stop=False)
            nc.tensor.matmul(out=G[:], lhsT=pk[0:17, C_XT : C_XT + STEPS],
                             rhs=pk[0:17, C_WX : C_WX + G3], start=False, stop=False)

            # transposes of the gathered/computed embeddings, then their matmuls
            p_ptT = ps.tile([4, H], F32, space="PSUM")
            nc.tensor.transpose(out=p_ptT[:], in_=st[:], identity=ident)
            nc.vector.tensor_copy(out=ptT[:], in_=p_ptT[:])
            p_ipT = ps.tile([4, H], F32, space="PSUM")
            nc.tensor.transpose(out=p_ipT[:], in_=ipf[:, :], identity=ident)
            nc.vector.tensor_copy(out=ipT[:], in_=p_ipT[:])
            nc.tensor.matmul(out=G[:], lhsT=ptT[:, 0:STEPS],
                             rhs=pk[0:4, C_WPT0 : C_WPT0 + G3], start=False, stop=False)
            nc.tensor.matmul(out=G[:], lhsT=ptT[:, STEPS : 2 * STEPS],
                             rhs=pk[0:4, C_WPT1 : C_WPT1 + G3], start=False, stop=False)
            nc.tensor.matmul(out=G[:], lhsT=ipT[:, 0:STEPS],
                             rhs=pk[0:4, C_WIP0 : C_WIP0 + G3], start=False, stop=False)
            nc.tensor.matmul(out=G[:], lhsT=ipT[:, STEPS : 2 * STEPS],
                             rhs=pk[0:4, C_WIP1 : C_WIP1 + G3], start=False, stop=True)

            # gates
            nc.scalar.activation(rz[:], G[:, 0 : 2 * HS],
                                 mybir.ActivationFunctionType.Sigmoid)
            r = rz[:, 0:HS]
            z = rz[:, HS : 2 * HS]
            nc.vector.scalar_tensor_tensor(
                out=t2[:], in0=r, scalar=1.0, in1=HN[:], op0=A.subtract, op1=A.mult)
            nc.vector.tensor_add(out=u[:], in0=G[:, 2 * HS : 3 * HS], in1=t2[:])
            nc.vector.tensor_scalar_add(out=zz[:], in0=z, scalar1=-1.0)
            nc.vector.tensor_mul(out=zh[:], in0=z, in1=H0B[:])
            nc.scalar.activation(n[:], u[:], mybir.ActivationFunctionType.Tanh)
            # o = z*h0 - n*(z-1) = n + z*(h0-n)
            nc.vector.tensor_mul(out=m[:], in0=n[:], in1=zz[:])
            nc.vector.tensor_sub(out=o[:], in0=zh[:], in1=m[:])

            nc.sync.dma_start(out=out_d[:, :], in_=o[:])

    nc.finalize()
    return nc


def make_in_maps(inputs):
    x = np.asarray(inputs["x"], dtype=np.float32)
    ipi = np.asarray(inputs["ip"], dtype=np.int32)
    pti = np.asarray(inputs["port"], dtype=np.int32)
    hid = np.asarray(inputs["hidden"], dtype=np.float32).reshape(H)
    ip_emb = np.ascontiguousarray(np.asarray(inputs["ip_emb"], dtype=np.float32))
    port_emb = np.ascontiguousarray(np.asarray(inputs["port_emb"], dtype=np.float32))
    W_ih = np.asarray(inputs["W_ih"], dtype=np.float32)
    W_hh = np.asarray(inputs["W_hh"], dtype=np.float32)
    b = np.asarray(inputs["b_ih"], dtype=np.float32) + np.asarray(
        inputs["b_hh"], dtype=np.float32
    )
    b_hh = np.asarray(inputs["b_hh"], dtype=np.float32)

    idx = np.ascontiguousarray(pti.T.reshape(H, 1))  # port[s,k] at partition k*64+s

    dv = np.zeros((H, 2 * V + 4), dtype=np.float32)
    dv[:, 0:V] = ip_emb[:, 0]
    dv[:, V : 2 * V] = np.arange(V, dtype=np.float32)
    dv[:, 2 * V : 2 * V + 4] = (
        ipi.reshape(STEPS, 2, 4).transpose(1, 0, 2).reshape(H, 4).astype(np.float32)
    )

    in_maps = []
    for c in range(N_CORES):
        sl = np.arange(c * HS, (c + 1) * HS)
        rows = np.concatenate([sl, H + sl, 2 * H + sl])
        pk = np.zeros((H, F_PK), dtype=np.float32)
        pk[0:H, C_WHT : C_WHT + G3] = W_hh[rows].T
        pk[0:17, C_WX : C_WX + G3] = W_ih[rows, 0:17].T
        pk[0:4, C_WIP0 : C_WIP0 + G3] = W_ih[rows, 17:21].T
        pk[0:4, C_WIP1 : C_WIP1 + G3] = W_ih[rows, 21:25].T
        pk[0:4, C_WPT0 : C_WPT0 + G3] = W_ih[rows, 25:29].T
        pk[0:4, C_WPT1 : C_WPT1 + G3] = W_ih[rows, 29:33].T
        pk[0:17, C_XT : C_XT + STEPS] = x.T
        pk[:, C_ID : C_ID + H] = np.eye(H, dtype=np.float32)
        pk[:, C_H] = hid
        pk[0, C_B : C_B + G3] = b[rows]
        pk[0, C_BN : C_BN + HS] = b_hh[2 * H + sl]
        pk[0, C_H0 : C_H0 + HS] = hid[sl]
        pk[0, C_ONE : C_ONE + STEPS] = 1.0
        in_maps.append(
            {"idx": idx, "dv": dv, "pk": pk, "port_emb": port_emb}
        )
    return in_maps


_NC = None


def run(inputs, trace=False):
    global _NC
    if _NC is None:
        _NC = build_nc()
    res = run_bass_kernel_spmd(_NC, make_in_maps(inputs), list(range(N_CORES)), trace=trace)
    outputs = np.concatenate([res.results[c]["out"] for c in range(N_CORES)], axis=1)
    new_hidden = np.ascontiguousarray(outputs[STEPS - 1].reshape(1, 1, H))
    return (outputs, new_hidden), res


def kernel(**inputs):
    (outputs, new_hidden), _ = run(inputs)
    return outputs, new_hidden


# revision 18
# speedup vs baseline: 1.0104x; 1.0104x over previous
"""Trainium2 Bass kernel for nn_Encoder_57380763074770.

GRU-cell encoder over 64 independent "steps":
  xi  = concat(x[64,17], ip_emb[ip].reshape(64,8), port_emb[port].reshape(64,8))
  G   = xi @ W_ih.T + h0 @ W_hh.T + (b_ih + b_hh)       # [64, 384]
  r, z = sigmoid(G_r), sigmoid(G_z)
  n   = tanh(G_n + (r - 1) * hn),  hn = h0 @ W_hh_n.T + b_hh_n
  out = n + z * (h0 - n)                                # [64, 128]

Sharding: H=128 hidden columns split 8 ways -> each core owns 16 columns of
every gate (48 rows of W_ih/W_hh) and computes out[:, 16c:16c+16].

Layout decisions (driven by the HW profile -- fixed costs dominate at this
size: ~0.6us per dma_start issue, ~1.5-2us DMA completion latency, ~1.1us
per indirect DMA on the Q7, ~1.3us per activation-table load):
- Weights/x ride HOST-TRANSPOSED (contraction-major) inside ONE packed
  [128, 625] f32 DMA, so the PE does no weight transposes at all.
- Port indices and ip indices ride in one tiny [128, 5] i32 DMA that lands
  first; the replicated 256-entry ip table rides on the Scalar-engine HWDGE
  in parallel.
- The 512 ip_emb lookups are computed on the VECTOR engine (128 partitions,
  partition j*64+s holds columns k=4j+g): int32 iota (generated on GpSimd)
  + one-hot is_equal + multiply + blocked 3D reduce.
- The port gather (70000x4 table) is ONE indirect DMA of 128 row lookups.
- G is accumulated in PSUM h-parts/bias/x first; only the final two
  matmuls wait on the DVE ip path.
- Activation tables pre-warmed against a memset scratch so both loads
  overlap the input DMAs.
"""

import numpy as np

import concourse.bacc as bacc
import concourse.bass as bass
import concourse.mybir as mybir
import concourse.tile as tile
from concourse.bass_utils import run_bass_kernel_spmd

STEPS = 64
H = 128
N_CORES = 8
HS = H // N_CORES       # hidden cols per core = 16
G3 = 3 * HS             # gate rows per core = 48

F32 = mybir.dt.float32
I32 = mybir.dt.int32

# packed params column layout ("pk", [128, F_PK]); all W blocks contraction-major
C_WHT = 0               # [0:128, 0:48]    W_hh slice, transposed
C_WX = 48               # [0:17, 48:96]    W_ih x-features, transposed
C_WIP0 = 96             # [0:4, 96:144]    W_ih ip k=0..3, transposed
C_WIP1 = 144            # [0:4, 144:192]   W_ih ip k=4..7, transposed
C_WPT0 = 192            # [0:4, 192:240]   W_ih port k=0, transposed
C_WPT1 = 240            # [0:4, 240:288]   W_ih port k=1, transposed
C_XT = 288              # [0:17, 288:352]  x transposed
C_ID = 352              # [0:128, 352:480] identity
C_H = 480               # [0:128, 480]     h0 column
C_B = 481               # [0, 481:529]     b_ih + b_hh slice
C_BN = 529              # [0, 529:545]     b_hh n-gate slice
C_H0 = 545              # [0, 545:561]     h0 slice for this core
C_ONE = 561             # [0, 561:625]     ones row
F_PK = 625

V = 256                 # ip table size


def build_nc():
    nc = bacc.Bacc(None)

    idx_d = nc.declare_dram_parameter("idx", [H, 1], I32, isOutput=False)
    dv_d = nc.declare_dram_parameter("dv", [H, 2 * V + 4], F32, isOutput=False)
    pk_d = nc.declare_dram_parameter("pk", [H, F_PK], F32, isOutput=False)
    pte_d = nc.declare_dram_parameter("port_emb", [70000, 4], F32, isOutput=False)
    out_d = nc.declare_dram_parameter("out", [STEPS, HS], F32, isOutput=True)

    with tile.TileContext(nc) as tc:
        with (
            tc.tile_pool(name="sb", bufs=1) as sb,
            tc.tile_pool(name="ps", bufs=1, space="PSUM") as ps,
        ):
            idx = sb.tile([H, 1], I32)
            dv = sb.tile([H, 2 * V + 4], F32)
            pk = sb.tile([H, F_PK], F32)
            st = sb.tile([H, 4], F32)
            wsrc = sb.tile([1, 1], F32)
            warm = sb.tile([1, 2], F32)
            pr = sb.tile([H, V], F32)
            ipf = sb.tile([H, 4], F32)
            ipT = sb.tile([4, H], F32)
            ptT = sb.tile([4, H], F32)
            rz = sb.tile([STEPS, 2 * HS], F32)
            t2 = sb.tile([STEPS, HS], F32)
            u = sb.tile([STEPS, HS], F32)
            n = sb.tile([STEPS, HS], F32)
            zz = sb.tile([STEPS, HS], F32)
            zh = sb.tile([STEPS, HS], F32)
            m = sb.tile([STEPS, HS], F32)
            o = sb.tile([STEPS, HS], F32)

            # idx + dv on Sync (they gate the gather and the DVE ip path);
            # pk rides the Scalar-engine HWDGE in parallel (PE prefix has slack)
            nc.sync.dma_start(out=idx[:], in_=idx_d[:, :])
            nc.scalar.dma_start(out=pk[:], in_=pk_d[:, :])
            nc.sync.dma_start(out=dv[:], in_=dv_d[:, :])

            # gpsimd: scratch memset + ONLY the port gather (clean Q7 state)
            nc.gpsimd.memset(wsrc[:], 0.25)
            nc.gpsimd.indirect_dma_start(
                out=st[:],
                out_offset=None,
                in_=pte_d[:, :],
                in_offset=bass.IndirectOffsetOnAxis(ap=idx[:, 0:1], axis=0),
            )

            # pre-warm both activation tables (reads only the memset scratch)
            nc.scalar.activation(warm[:, 0:1], wsrc[:],
                                 mybir.ActivationFunctionType.Tanh)
            nc.scalar.activation(warm[:, 1:2], wsrc[:],
                                 mybir.ActivationFunctionType.Sigmoid)

            # ip embedding on DVE (partition j*64+s holds k = 4j+g): per column
            # ONE fused op: (iota == idx_k) * table, accumulated to a scalar
            A = mybir.AluOpType
            iof = dv[:, V : 2 * V]
            emb = dv[:, 0:V]
            for k in range(4):
                nc.vector.scalar_tensor_tensor(
                    out=pr[:, :], in0=iof, scalar=dv[:, 2 * V + k : 2 * V + k + 1],
                    in1=emb, op0=A.is_equal, op1=A.mult,
                    accum_out=ipf[:, k : k + 1])

            ident = pk[:, C_ID : C_ID + H]
            hcol_b = pk[:, C_H : C_H + 1].to_broadcast([H, STEPS])
            ones = pk[0:1, C_ONE : C_ONE + STEPS]

            # h-dependent matmuls (all inputs direct from pk)
            HN = ps.tile([STEPS, HS], F32, space="PSUM")
            nc.tensor.matmul(out=HN[:], lhsT=hcol_b,
                             rhs=pk[0:H, C_WHT + 2 * HS : C_WHT + 3 * HS],
                             start=True, stop=False)
            nc.tensor.matmul(out=HN[:], lhsT=ones, rhs=pk[0:1, C_BN : C_BN + HS],
                             start=False, stop=True)

            H0B = ps.tile([STEPS, HS], F32, space="PSUM")
            nc.tensor.matmul(out=H0B[:], lhsT=ones, rhs=pk[0:1, C_H0 : C_H0 + HS],
                             start=True, stop=True)

            # G accumulation: everything not gather/ip-dependent first
            G = ps.tile([STEPS, G3], F32, space="PSUM")
            nc.tensor.matmul(out=G[:], lhsT=hcol_b, rhs=pk[0:H, C_WHT : C_WHT + G3],
                             start=True, stop=False)
            nc.tensor.matmul(out=G[:], lhsT=ones, rhs=pk[0:1, C_B : C_B + G3],
                             start=False, stop=False)
            nc.tensor.matmul(out=G[:], lhsT=pk[0:17, C_XT : C_XT + STEPS],
                             rhs=pk[0:17, C_WX : C_WX + G3], start=False, stop=False)

            # transposes of the gathered/computed embeddings, then their matmuls
            p_ptT = ps.tile([4, H], F32, space="PSUM")
            nc.tensor.transpose(out=p_ptT[:], in_=st[:], identity=ident)
            nc.vector.tensor_copy(out=ptT[:], in_=p_ptT[:])
            p_ipT = ps.tile([4, H], F32, space="PSUM")
            nc.tensor.transpose(out=p_ipT[:], in_=ipf[:, :], identity=ident)
            nc.vector.tensor_copy(out=ipT[:], in_=p_ipT[:])
            nc.tensor.matmul(out=G[:], lhsT=ptT[:, 0:STEPS],
                             rhs=pk[0:4, C_WPT0 : C_WPT0 + G3], start=False, stop=False)
            nc.tensor.matmul(out=G[:], lhsT=ptT[:, STEPS : 2 * STEPS],
                             rhs=pk[0:4, C_WPT1 : C_WPT1 + G3], start=False, stop=False)
            nc.tensor.matmul(out=G[:], lhsT=ipT[:, 0:STEPS],
                             rhs=pk[0:4, C_WIP0 : C_WIP0 + G3], start=False, stop=False)
            nc.tensor.matmul(out=G[:], lhsT=ipT[:, STEPS : 2 * STEPS],
                             rhs=pk[0:4, C_WIP1 : C_WIP1 + G3], start=False, stop=True)

            # gates
            nc.scalar.activation(rz[:], G[:, 0 : 2 * HS],
                                 mybir.ActivationFunctionType.Sigmoid)
            r = rz[:, 0:HS]
            z = rz[:, HS : 2 * HS]
            nc.vector.scalar_tensor_tensor(
                out=t2[:], in0=r, scalar=1.0, in1=HN[:], op0=A.subtract, op1=A.mult)
            nc.vector.tensor_add(out=u[:], in0=G[:, 2 * HS : 3 * HS], in1=t2[:])
            nc.vector.tensor_scalar_add(out=zz[:], in0=z, scalar1=-1.0)
            nc.vector.tensor_mul(out=zh[:], in0=z, in1=H0B[:])
            nc.scalar.activation(n[:], u[:], mybir.ActivationFunctionType.Tanh)
            # o = z*h0 - n*(z-1) = n + z*(h0-n)
            nc.vector.tensor_mul(out=m[:], in0=n[:], in1=zz[:])
            nc.vector.tensor_sub(out=o[:], in0=zh[:], in1=m[:])

            nc.sync.dma_start(out=out_d[:, :], in_=o[:])

    nc.finalize()
    return nc


def make_in_maps(inputs):
    x = np.asarray(inputs["x"], dtype=np.float32)
    ipi = np.asarray(inputs["ip"], dtype=np.int32)
    pti = np.asarray(inputs["port"], dtype=np.int32)
    hid = np.asarray(inputs["hidden"], dtype=np.float32).reshape(H)
    ip_emb = np.ascontiguousarray(np.asarray(inputs["ip_emb"], dtype=np.float32))
    port_emb = np.ascontiguousarray(np.asarray(inputs["port_emb"], dtype=np.float32))
    W_ih = np.asarray(inputs["W_ih"], dtype=np.float32)
    W_hh = np.asarray(inputs["W_hh"], dtype=np.float32)
    b = np.asarray(inputs["b_ih"], dtype=np.float32) + np.asarray(
        inputs["b_hh"], dtype=np.float32
    )
    b_hh = np.asarray(inputs["b_hh"], dtype=np.float32)

    idx = np.ascontiguousarray(pti.T.reshape(H, 1))  # port[s,k] at partition k*64+s

    dv = np.zeros((H, 2 * V + 4), dtype=np.float32)
    dv[:, 0:V] = ip_emb[:, 0]
    dv[:, V : 2 * V] = np.arange(V, dtype=np.float32)
    dv[:, 2 * V : 2 * V + 4] = (
        ipi.reshape(STEPS, 2, 4).transpose(1, 0, 2).reshape(H, 4).astype(np.float32)
    )

    in_maps = []
    for c in range(N_CORES):
        sl = np.arange(c * HS, (c + 1) * HS)
        rows = np.concatenate([sl, H + sl, 2 * H + sl])
        pk = np.zeros((H, F_PK), dtype=np.float32)
        pk[0:H, C_WHT : C_WHT + G3] = W_hh[rows].T
        pk[0:17, C_WX : C_WX + G3] = W_ih[rows, 0:17].T
        pk[0:4, C_WIP0 : C_WIP0 + G3] = W_ih[rows, 17:21].T
        pk[0:4, C_WIP1 : C_WIP1 + G3] = W_ih[rows, 21:25].T
        pk[0:4, C_WPT0 : C_WPT0 + G3] = W_ih[rows, 25:29].T
        pk[0:4, C_WPT1 : C_WPT1 + G3] = W_ih[rows, 29:33].T
        pk[0:17, C_XT : C_XT + STEPS] = x.T
        pk[:, C_ID : C_ID + H] = np.eye(H, dtype=np.float32)
        pk[:, C_H] = hid
        pk[0, C_B : C_B + G3] = b[rows]
        pk[0, C_BN : C_BN + HS] = b_hh[2 * H + sl]
        pk[0, C_H0 : C_H0 + HS] = hid[sl]
        pk[0, C_ONE : C_ONE + STEPS] = 1.0
        in_maps.append(
            {"idx": idx, "dv": dv, "pk": pk, "port_emb": port_emb}
        )
    return in_maps


_NC = None


def run(inputs, trace=False):
    global _NC
    if _NC is None:
        _NC = build_nc()
    res = run_bass_kernel_spmd(_NC, make_in_maps(inputs), list(range(N_CORES)), trace=trace)
    outputs = np.concatenate([res.results[c]["out"] for c in range(N_CORES)], axis=1)
    new_hidden = np.ascontiguousarray(outputs[STEPS - 1].reshape(1, 1, H))
    return (outputs, new_hidden), res


def kernel(**inputs):
    (outputs, new_hidden), _ = run(inputs)
    return outputs, new_hidden


# revision 19
# speedup vs baseline: 1.0257x; 1.0151x over previous
"""Trainium2 Bass kernel for nn_Encoder_57380763074770.

GRU-cell encoder over 64 independent "steps":
  xi  = concat(x[64,17], ip_emb[ip].reshape(64,8), port_emb[port].reshape(64,8))
  G   = xi @ W_ih.T + h0 @ W_hh.T + (b_ih + b_hh)       # [64, 384]
  r, z = sigmoid(G_r), sigmoid(G_z)
  n   = tanh(G_n + (r - 1) * hn),  hn = h0 @ W_hh_n.T + b_hh_n
  out = n + z * (h0 - n)                                # [64, 128]

Sharding: H=128 hidden columns split 8 ways -> each core owns 16 columns of
every gate (48 rows of W_ih/W_hh) and computes out[:, 16c:16c+16].

Layout decisions (driven by the HW profile -- fixed costs dominate at this
size: ~0.6us per dma_start issue, ~1.5-2us DMA completion latency, ~1.1us
per indirect DMA on the Q7, ~1.3us per activation-table load):
- Weights/x ride HOST-TRANSPOSED (contraction-major) inside ONE packed
  [128, 625] f32 DMA, so the PE does no weight transposes at all.
- Port indices and ip indices ride in one tiny [128, 5] i32 DMA that lands
  first; the replicated 256-entry ip table rides on the Scalar-engine HWDGE
  in parallel.
- The 512 ip_emb lookups are computed on the VECTOR engine (128 partitions,
  partition j*64+s holds columns k=4j+g): int32 iota (generated on GpSimd)
  + one-hot is_equal + multiply + blocked 3D reduce.
- The port gather (70000x4 table) is ONE indirect DMA of 128 row lookups.
- G is accumulated in PSUM h-parts/bias/x first; only the final two
  matmuls wait on the DVE ip path.
- Activation tables pre-warmed against a memset scratch so both loads
  overlap the input DMAs.
"""

import numpy as np

import concourse.bacc as bacc
import concourse.bass as bass
import concourse.mybir as mybir
import concourse.tile as tile
from concourse.bass_utils import run_bass_kernel_spmd

STEPS = 64
H = 128
N_CORES = 8
HS = H // N_CORES       # hidden cols per core = 16
G3 = 3 * HS             # gate rows per core = 48

F32 = mybir.dt.float32
I32 = mybir.dt.int32

# packed params column layout ("pk", [128, F_PK]); all W blocks contraction-major
C_WHT = 0               # [0:128, 0:48]    W_hh slice, transposed
C_WX = 48               # [0:17, 48:96]    W_ih x-features, transposed
C_WIP0 = 96             # [0:4, 96:144]    W_ih ip k=0..3, transposed
C_WIP1 = 144            # [0:4, 144:192]   W_ih ip k=4..7, transposed
C_WPT0 = 192            # [0:4, 192:240]   W_ih port k=0, transposed
C_WPT1 = 240            # [0:4, 240:288]   W_ih port k=1, transposed
C_XT = 288              # [0:17, 288:352]  x transposed
C_ID = 352              # [0:128, 352:480] identity
C_H = 480               # [0:128, 480]     h0 column
C_B = 481               # [0, 481:529]     b_ih + b_hh slice
C_BN = 529              # [0, 529:545]     b_hh n-gate slice
C_H0 = 545              # [0, 545:561]     h0 slice for this core
C_ONE = 561             # [0, 561:625]     ones row
F_PK = 625

V = 256                 # ip table size


def build_nc():
    nc = bacc.Bacc(None)

    dv_d = nc.declare_dram_parameter("dv", [H, 2 * V + 5], F32, isOutput=False)
    pk_d = nc.declare_dram_parameter("pk", [H, F_PK], F32, isOutput=False)
    pte_d = nc.declare_dram_parameter("port_emb", [70000, 4], F32, isOutput=False)
    out_d = nc.declare_dram_parameter("out", [STEPS, HS], F32, isOutput=True)

    with tile.TileContext(nc) as tc:
        with (
            tc.tile_pool(name="sb", bufs=1) as sb,
            tc.tile_pool(name="ps", bufs=1, space="PSUM") as ps,
        ):
            dv = sb.tile([H, 2 * V + 5], F32)
            pk = sb.tile([H, F_PK], F32)
            st = sb.tile([H, 4], F32)
            wsrc = sb.tile([1, 1], F32)
            warm = sb.tile([1, 2], F32)
            pr = sb.tile([H, V], F32)
            ipf = sb.tile([H, 4], F32)
            ipT = sb.tile([4, H], F32)
            ptT = sb.tile([4, H], F32)
            rz = sb.tile([STEPS, 2 * HS], F32)
            t2 = sb.tile([STEPS, HS], F32)
            u = sb.tile([STEPS, HS], F32)
            n = sb.tile([STEPS, HS], F32)
            zz = sb.tile([STEPS, HS], F32)
            zh = sb.tile([STEPS, HS], F32)
            m = sb.tile([STEPS, HS], F32)
            o = sb.tile([STEPS, HS], F32)

            # TWO input streams: dv on Sync (gates the gather AND the DVE ip
            # path -- port index rides as a bitcast i32 column); pk on the
            # Scalar-engine HWDGE in parallel (PE prefix has slack)
            nc.sync.dma_start(out=dv[:], in_=dv_d[:, :])
            nc.scalar.dma_start(out=pk[:], in_=pk_d[:, :])

            # gpsimd: scratch memset + ONLY the port gather (clean Q7 state)
            nc.gpsimd.memset(wsrc[:], 0.25)
            nc.gpsimd.indirect_dma_start(
                out=st[:],
                out_offset=None,
                in_=pte_d[:, :],
                in_offset=bass.IndirectOffsetOnAxis(
                    ap=dv[:, 2 * V + 4 : 2 * V + 5].bitcast(I32), axis=0),
            )

            # pre-warm both activation tables (reads only the memset scratch)
            nc.scalar.activation(warm[:, 0:1], wsrc[:],
                                 mybir.ActivationFunctionType.Tanh)
            nc.scalar.activation(warm[:, 1:2], wsrc[:],
                                 mybir.ActivationFunctionType.Sigmoid)

            # ip embedding on DVE (partition j*64+s holds k = 4j+g): per column
            # ONE fused op: (iota == idx_k) * table, accumulated to a scalar
            A = mybir.AluOpType
            iof = dv[:, V : 2 * V]
            emb = dv[:, 0:V]
            for k in range(4):
                nc.vector.scalar_tensor_tensor(
                    out=pr[:, :], in0=iof, scalar=dv[:, 2 * V + k : 2 * V + k + 1],
                    in1=emb, op0=A.is_equal, op1=A.mult,
                    accum_out=ipf[:, k : k + 1])

            ident = pk[:, C_ID : C_ID + H]
            hcol_b = pk[:, C_H : C_H + 1].to_broadcast([H, STEPS])
            ones = pk[0:1, C_ONE : C_ONE + STEPS]

            # h-dependent matmuls (all inputs direct from pk)
            HN = ps.tile([STEPS, HS], F32, space="PSUM")
            nc.tensor.matmul(out=HN[:], lhsT=hcol_b,
                             rhs=pk[0:H, C_WHT + 2 * HS : C_WHT + 3 * HS],
                             start=True, stop=False)
            nc.tensor.matmul(out=HN[:], lhsT=ones, rhs=pk[0:1, C_BN : C_BN + HS],
                             start=False, stop=True)

            H0B = ps.tile([STEPS, HS], F32, space="PSUM")
            nc.tensor.matmul(out=H0B[:], lhsT=ones, rhs=pk[0:1, C_H0 : C_H0 + HS],
                             start=True, stop=True)

            # G accumulation: everything not gather/ip-dependent first
            G = ps.tile([STEPS, G3], F32, space="PSUM")
            nc.tensor.matmul(out=G[:], lhsT=hcol_b, rhs=pk[0:H, C_WHT : C_WHT + G3],
                             start=True, stop=False)
            nc.tensor.matmul(out=G[:], lhsT=ones, rhs=pk[0:1, C_B : C_B + G3],
                             start=False, stop=False)
            nc.tensor.matmul(out=G[:], lhsT=pk[0:17, C_XT : C_XT + STEPS],
                             rhs=pk[0:17, C_WX : C_WX + G3], start=False, stop=False)

            # transposes of the gathered/computed embeddings, then their matmuls
            p_ptT = ps.tile([4, H], F32, space="PSUM")
            nc.tensor.transpose(out=p_ptT[:], in_=st[:], identity=ident)
            nc.vector.tensor_copy(out=ptT[:], in_=p_ptT[:])
            p_ipT = ps.tile([4, H], F32, space="PSUM")
            nc.tensor.transpose(out=p_ipT[:], in_=ipf[:, :], identity=ident)
            nc.vector.tensor_copy(out=ipT[:], in_=p_ipT[:])
            nc.tensor.matmul(out=G[:], lhsT=ptT[:, 0:STEPS],
                             rhs=pk[0:4, C_WPT0 : C_WPT0 + G3], start=False, stop=False)
            nc.tensor.matmul(out=G[:], lhsT=ptT[:, STEPS : 2 * STEPS],
                             rhs=pk[0:4, C_WPT1 : C_WPT1 + G3], start=False, stop=False)
            nc.tensor.matmul(out=G[:], lhsT=ipT[:, 0:STEPS],
                             rhs=pk[0:4, C_WIP0 : C_WIP0 + G3], start=False, stop=False)
            nc.tensor.matmul(out=G[:], lhsT=ipT[:, STEPS : 2 * STEPS],
                             rhs=pk[0:4, C_WIP1 : C_WIP1 + G3], start=False, stop=True)

            # gates
            nc.scalar.activation(rz[:], G[:, 0 : 2 * HS],
                                 mybir.ActivationFunctionType.Sigmoid)
            r = rz[:, 0:HS]
            z = rz[:, HS : 2 * HS]
            nc.vector.scalar_tensor_tensor(
                out=t2[:], in0=r, scalar=1.0, in1=HN[:], op0=A.subtract, op1=A.mult)
            nc.vector.tensor_add(out=u[:], in0=G[:, 2 * HS : 3 * HS], in1=t2[:])
            nc.vector.tensor_scalar_add(out=zz[:], in0=z, scalar1=-1.0)
            nc.vector.tensor_mul(out=zh[:], in0=z, in1=H0B[:])
            nc.scalar.activation(n[:], u[:], mybir.ActivationFunctionType.Tanh)
            # o = z*h0 - n*(z-1) = n + z*(h0-n)
            nc.vector.tensor_mul(out=m[:], in0=n[:], in1=zz[:])
            nc.vector.tensor_sub(out=o[:], in0=zh[:], in1=m[:])

            nc.sync.dma_start(out=out_d[:, :], in_=o[:])

    nc.finalize()
    return nc


def make_in_maps(inputs):
    x = np.asarray(inputs["x"], dtype=np.float32)
    ipi = np.asarray(inputs["ip"], dtype=np.int32)
    pti = np.asarray(inputs["port"], dtype=np.int32)
    hid = np.asarray(inputs["hidden"], dtype=np.float32).reshape(H)
    ip_emb = np.ascontiguousarray(np.asarray(inputs["ip_emb"], dtype=np.float32))
    port_emb = np.ascontiguousarray(np.asarray(inputs["port_emb"], dtype=np.float32))
    W_ih = np.asarray(inputs["W_ih"], dtype=np.float32)
    W_hh = np.asarray(inputs["W_hh"], dtype=np.float32)
    b = np.asarray(inputs["b_ih"], dtype=np.float32) + np.asarray(
        inputs["b_hh"], dtype=np.float32
    )
    b_hh = np.asarray(inputs["b_hh"], dtype=np.float32)

    dv = np.zeros((H, 2 * V + 5), dtype=np.float32)
    dv[:, 0:V] = ip_emb[:, 0]
    dv[:, V : 2 * V] = np.arange(V, dtype=np.float32)
    dv[:, 2 * V : 2 * V + 4] = (
        ipi.reshape(STEPS, 2, 4).transpose(1, 0, 2).reshape(H, 4).astype(np.float32)
    )
    # port[s,k] at partition k*64+s, int32 bits in an f32 column
    dv[:, 2 * V + 4] = pti.T.reshape(H).astype(np.int32).view(np.float32)

    in_maps = []
    for c in range(N_CORES):
        sl = np.arange(c * HS, (c + 1) * HS)
        rows = np.concatenate([sl, H + sl, 2 * H + sl])
        pk = np.zeros((H, F_PK), dtype=np.float32)
        pk[0:H, C_WHT : C_WHT + G3] = W_hh[rows].T
        pk[0:17, C_WX : C_WX + G3] = W_ih[rows, 0:17].T
        pk[0:4, C_WIP0 : C_WIP0 + G3] = W_ih[rows, 17:21].T
        pk[0:4, C_WIP1 : C_WIP1 + G3] = W_ih[rows, 21:25].T
        pk[0:4, C_WPT0 : C_WPT0 + G3] = W_ih[rows, 25:29].T
        pk[0:4, C_WPT1 : C_WPT1 + G3] = W_ih[rows, 29:33].T
        pk[0:17, C_XT : C_XT + STEPS] = x.T
        pk[:, C_ID : C_ID + H] = np.eye(H, dtype=np.float32)
        pk[:, C_H] = hid
        pk[0, C_B : C_B + G3] = b[rows]
        pk[0, C_BN : C_BN + HS] = b_hh[2 * H + sl]
        pk[0, C_H0 : C_H0 + HS] = hid[sl]
        pk[0, C_ONE : C_ONE + STEPS] = 1.0
        in_maps.append({"dv": dv, "pk": pk, "port_emb": port_emb})
    return in_maps


_NC = None


def run(inputs, trace=False):
    global _NC
    if _NC is None:
        _NC = build_nc()
    res = run_bass_kernel_spmd(_NC, make_in_maps(inputs), list(range(N_CORES)), trace=trace)
    outputs = np.concatenate([res.results[c]["out"] for c in range(N_CORES)], axis=1)
    new_hidden = np.ascontiguousarray(outputs[STEPS - 1].reshape(1, 1, H))
    return (outputs, new_hidden), res


def kernel(**inputs):
    (outputs, new_hidden), _ = run(inputs)
    return outputs, new_hidden


# revision 20
# speedup vs baseline: 1.0602x; 1.0336x over previous
"""Trainium2 Bass kernel for nn_Encoder_57380763074770.

GRU-cell encoder over 64 independent "steps":
  xi  = concat(x[64,17], ip_emb[ip].reshape(64,8), port_emb[port].reshape(64,8))
  G   = xi @ W_ih.T + h0 @ W_hh.T + (b_ih + b_hh)       # [64, 384]
  r, z = sigmoid(G_r), sigmoid(G_z)
  n   = tanh(G_n + (r - 1) * hn),  hn = h0 @ W_hh_n.T + b_hh_n
  out = n + z * (h0 - n)                                # [64, 128]

Sharding: H=128 hidden columns split 8 ways -> each core owns 16 columns of
every gate (48 rows of W_ih/W_hh) and computes out[:, 16c:16c+16].

Layout decisions (driven by the HW profile -- fixed costs dominate at this
size: ~0.6us per dma_start issue, ~1.5-2us DMA completion latency, ~1.1us
per indirect DMA on the Q7, ~1.3us per activation-table load):
- Weights/x ride HOST-TRANSPOSED (contraction-major) inside ONE packed
  [128, 625] f32 DMA, so the PE does no weight transposes at all.
- Port indices and ip indices ride in one tiny [128, 5] i32 DMA that lands
  first; the replicated 256-entry ip table rides on the Scalar-engine HWDGE
  in parallel.
- The 512 ip_emb lookups are computed on the VECTOR engine (128 partitions,
  partition j*64+s holds columns k=4j+g): int32 iota (generated on GpSimd)
  + one-hot is_equal + multiply + blocked 3D reduce.
- The port gather (70000x4 table) is ONE indirect DMA of 128 row lookups.
- G is accumulated in PSUM h-parts/bias/x first; only the final two
  matmuls wait on the DVE ip path.
- Activation tables pre-warmed against a memset scratch so both loads
  overlap the input DMAs.
"""

import numpy as np

import concourse.bacc as bacc
import concourse.bass as bass
import concourse.mybir as mybir
import concourse.tile as tile
from concourse.bass_utils import run_bass_kernel_spmd

STEPS = 64
H = 128
N_CORES = 8
HS = H // N_CORES       # hidden cols per core = 16
G3 = 3 * HS             # gate rows per core = 48

F32 = mybir.dt.float32
I32 = mybir.dt.int32
BF16 = mybir.dt.bfloat16

# packed params column layout ("pk", [128, F_PK]); all W blocks contraction-major
C_WHT = 0               # [0:128, 0:48]    W_hh slice, transposed
C_WX = 48               # [0:17, 48:96]    W_ih x-features, transposed
C_WIP0 = 96             # [0:4, 96:120]    W_ih ip k=0..3, transposed, bf16-packed
C_WIP1 = 120            # [0:4, 120:144]   W_ih ip k=4..7, transposed, bf16-packed
C_WPT0 = 144            # [0:4, 144:168]   W_ih port k=0, transposed, bf16-packed
C_WPT1 = 168            # [0:4, 168:192]   W_ih port k=1, transposed, bf16-packed
C_XT = 288              # [0:17, 288:352]  x transposed
C_ID = 352              # [0:128, 352:480] identity
C_H = 480               # [0:128, 480]     h0 column
C_B = 481               # [0, 481:529]     b_ih + b_hh slice
C_BN = 529              # [0, 529:545]     b_hh n-gate slice
C_H0 = 545              # [0, 545:561]     h0 slice for this core
C_ONE = 561             # [0, 561:625]     ones row
F_PK = 625

V = 256                 # ip table size


def build_nc():
    nc = bacc.Bacc(None)

    dv_d = nc.declare_dram_parameter("dv", [H, 2 * V + 5], F32, isOutput=False)
    pk_d = nc.declare_dram_parameter("pk", [H, F_PK], F32, isOutput=False)
    pte_d = nc.declare_dram_parameter("port_emb", [70000, 4], F32, isOutput=False)
    out_d = nc.declare_dram_parameter("out", [STEPS, HS], F32, isOutput=True)

    with tile.TileContext(nc) as tc:
        with (
            tc.tile_pool(name="sb", bufs=1) as sb,
            tc.tile_pool(name="ps", bufs=1, space="PSUM") as ps,
        ):
            dv = sb.tile([H, 2 * V + 5], F32)
            pk = sb.tile([H, F_PK], F32)
            st = sb.tile([H, 4], F32)
            wsrc = sb.tile([1, 1], F32)
            warm = sb.tile([1, 2], F32)
            pr = sb.tile([H, V], F32)
            ipf = sb.tile([H, 4], F32)
            ipT = sb.tile([4, H], BF16)
            ptT = sb.tile([4, H], BF16)
            rz = sb.tile([STEPS, 2 * HS], F32)
            t2 = sb.tile([STEPS, HS], F32)
            u = sb.tile([STEPS, HS], F32)
            n = sb.tile([STEPS, HS], F32)
            zz = sb.tile([STEPS, HS], F32)
            zh = sb.tile([STEPS, HS], F32)
            m = sb.tile([STEPS, HS], F32)
            o = sb.tile([STEPS, HS], F32)

            # TWO input streams: dv on Sync (gates the gather AND the DVE ip
            # path -- port index rides as a bitcast i32 column); pk on the
            # Scalar-engine HWDGE in parallel (PE prefix has slack)
            nc.sync.dma_start(out=dv[:], in_=dv_d[:, :])
            nc.scalar.dma_start(out=pk[:], in_=pk_d[:, :])

            # gpsimd: scratch memset + ONLY the port gather (clean Q7 state)
            nc.gpsimd.memset(wsrc[:], 0.25)
            nc.gpsimd.indirect_dma_start(
                out=st[:],
                out_offset=None,
                in_=pte_d[:, :],
                in_offset=bass.IndirectOffsetOnAxis(
                    ap=dv[:, 2 * V + 4 : 2 * V + 5].bitcast(I32), axis=0),
            )

            # pre-warm both activation tables (reads only the memset scratch)
            nc.scalar.activation(warm[:, 0:1], wsrc[:],
                                 mybir.ActivationFunctionType.Tanh)
            nc.scalar.activation(warm[:, 1:2], wsrc[:],
                                 mybir.ActivationFunctionType.Sigmoid)

            # ip embedding on DVE (partition j*64+s holds k = 4j+g): per column
            # ONE fused op: (iota == idx_k) * table, accumulated to a scalar
            A = mybir.AluOpType
            iof = dv[:, V : 2 * V]
            emb = dv[:, 0:V]
            for k in range(4):
                nc.vector.scalar_tensor_tensor(
                    out=pr[:, :], in0=iof, scalar=dv[:, 2 * V + k : 2 * V + k + 1],
                    in1=emb, op0=A.is_equal, op1=A.mult,
                    accum_out=ipf[:, k : k + 1])

            ident = pk[:, C_ID : C_ID + H]
            hcol_b = pk[:, C_H : C_H + 1].to_broadcast([H, STEPS])
            ones = pk[0:1, C_ONE : C_ONE + STEPS]

            # h-dependent matmuls (all inputs direct from pk)
            HN = ps.tile([STEPS, HS], F32, space="PSUM")
            nc.tensor.matmul(out=HN[:], lhsT=hcol_b,
                             rhs=pk[0:H, C_WHT + 2 * HS : C_WHT + 3 * HS],
                             start=True, stop=False)
            nc.tensor.matmul(out=HN[:], lhsT=ones, rhs=pk[0:1, C_BN : C_BN + HS],
                             start=False, stop=True)

            H0B = ps.tile([STEPS, HS], F32, space="PSUM")
            nc.tensor.matmul(out=H0B[:], lhsT=ones, rhs=pk[0:1, C_H0 : C_H0 + HS],
                             start=True, stop=True)

            # G accumulation: everything not gather/ip-dependent first
            G = ps.tile([STEPS, G3], F32, space="PSUM")
            nc.tensor.matmul(out=G[:], lhsT=hcol_b, rhs=pk[0:H, C_WHT : C_WHT + G3],
                             start=True, stop=False)
            nc.tensor.matmul(out=G[:], lhsT=ones, rhs=pk[0:1, C_B : C_B + G3],
                             start=False, stop=False)
            nc.tensor.matmul(out=G[:], lhsT=pk[0:17, C_XT : C_XT + STEPS],
                             rhs=pk[0:17, C_WX : C_WX + G3], start=False, stop=False)

            # transposes of the gathered/computed embeddings, then their matmuls
            p_ptT = ps.tile([4, H], F32, space="PSUM")
            nc.tensor.transpose(out=p_ptT[:], in_=st[:], identity=ident)
            nc.vector.tensor_copy(out=ptT[:], in_=p_ptT[:])
            p_ipT = ps.tile([4, H], F32, space="PSUM")
            nc.tensor.transpose(out=p_ipT[:], in_=ipf[:, :], identity=ident)
            nc.vector.tensor_copy(out=ipT[:], in_=p_ipT[:])
            nc.tensor.matmul(out=G[:], lhsT=ptT[:, 0:STEPS],
                             rhs=pk[0:4, C_WPT0 : C_WPT0 + G3 // 2].bitcast(BF16),
                             start=False, stop=False)
            nc.tensor.matmul(out=G[:], lhsT=ptT[:, STEPS : 2 * STEPS],
                             rhs=pk[0:4, C_WPT1 : C_WPT1 + G3 // 2].bitcast(BF16),
                             start=False, stop=False)
            nc.tensor.matmul(out=G[:], lhsT=ipT[:, 0:STEPS],
                             rhs=pk[0:4, C_WIP0 : C_WIP0 + G3 // 2].bitcast(BF16),
                             start=False, stop=False)
            nc.tensor.matmul(out=G[:], lhsT=ipT[:, STEPS : 2 * STEPS],
                             rhs=pk[0:4, C_WIP1 : C_WIP1 + G3 // 2].bitcast(BF16),
                             start=False, stop=True)

            # gates
            nc.scalar.activation(rz[:], G[:, 0 : 2 * HS],
                                 mybir.ActivationFunctionType.Sigmoid)
            r = rz[:, 0:HS]
            z = rz[:, HS : 2 * HS]
            nc.vector.scalar_tensor_tensor(
                out=t2[:], in0=r, scalar=1.0, in1=HN[:], op0=A.subtract, op1=A.mult)
            nc.vector.tensor_add(out=u[:], in0=G[:, 2 * HS : 3 * HS], in1=t2[:])
            nc.vector.tensor_scalar_add(out=zz[:], in0=z, scalar1=-1.0)
            nc.vector.tensor_mul(out=zh[:], in0=z, in1=H0B[:])
            nc.scalar.activation(n[:], u[:], mybir.ActivationFunctionType.Tanh)
            # o = z*h0 - n*(z-1) = n + z*(h0-n)
            nc.vector.tensor_mul(out=m[:], in0=n[:], in1=zz[:])
            nc.vector.tensor_sub(out=o[:], in0=zh[:], in1=m[:])

            nc.sync.dma_start(out=out_d[:, :], in_=o[:])

    nc.finalize()
    return nc


def make_in_maps(inputs):
    x = np.asarray(inputs["x"], dtype=np.float32)
    ipi = np.asarray(inputs["ip"], dtype=np.int32)
    pti = np.asarray(inputs["port"], dtype=np.int32)
    hid = np.asarray(inputs["hidden"], dtype=np.float32).reshape(H)
    ip_emb = np.ascontiguousarray(np.asarray(inputs["ip_emb"], dtype=np.float32))
    port_emb = np.ascontiguousarray(np.asarray(inputs["port_emb"], dtype=np.float32))
    W_ih = np.asarray(inputs["W_ih"], dtype=np.float32)
    W_hh = np.asarray(inputs["W_hh"], dtype=np.float32)
    b = np.asarray(inputs["b_ih"], dtype=np.float32) + np.asarray(
        inputs["b_hh"], dtype=np.float32
    )
    b_hh = np.asarray(inputs["b_hh"], dtype=np.float32)

    dv = np.zeros((H, 2 * V + 5), dtype=np.float32)
    dv[:, 0:V] = ip_emb[:, 0]
    dv[:, V : 2 * V] = np.arange(V, dtype=np.float32)
    dv[:, 2 * V : 2 * V + 4] = (
        ipi.reshape(STEPS, 2, 4).transpose(1, 0, 2).reshape(H, 4).astype(np.float32)
    )
    # port[s,k] at partition k*64+s, int32 bits in an f32 column
    dv[:, 2 * V + 4] = pti.T.reshape(H).astype(np.int32).view(np.float32)

    in_maps = []
    for c in range(N_CORES):
        sl = np.arange(c * HS, (c + 1) * HS)
        rows = np.concatenate([sl, H + sl, 2 * H + sl])
        pk = np.zeros((H, F_PK), dtype=np.float32)
        pk[0:H, C_WHT : C_WHT + G3] = W_hh[rows].T
        pk[0:17, C_WX : C_WX + G3] = W_ih[rows, 0:17].T
        def bf16pack(w):  # [4, 48] f32 -> [4, 24] f32 carrying bf16 pairs
            b = ((w.view(np.uint32) + 0x8000) >> 16).astype(np.uint16)
            return b.reshape(4, 24, 2).view(np.uint32).reshape(4, 24).view(np.float32)

        pk[0:4, C_WIP0 : C_WIP0 + G3 // 2] = bf16pack(W_ih[rows, 17:21].T.copy())
        pk[0:4, C_WIP1 : C_WIP1 + G3 // 2] = bf16pack(W_ih[rows, 21:25].T.copy())
        pk[0:4, C_WPT0 : C_WPT0 + G3 // 2] = bf16pack(W_ih[rows, 25:29].T.copy())
        pk[0:4, C_WPT1 : C_WPT1 + G3 // 2] = bf16pack(W_ih[rows, 29:33].T.copy())
        pk[0:17, C_XT : C_XT + STEPS] = x.T
        pk[:, C_ID : C_ID + H] = np.eye(H, dtype=np.float32)
        pk[:, C_H] = hid
        pk[0, C_B : C_B + G3] = b[rows]
        pk[0, C_BN : C_BN + HS] = b_hh[2 * H + sl]
        pk[0, C_H0 : C_H0 + HS] = hid[sl]
        pk[0, C_ONE : C_ONE + STEPS] = 1.0
        in_maps.append({"dv": dv, "pk": pk, "port_emb": port_emb})
    return in_maps


_NC = None


def run(inputs, trace=False):
    global _NC
    if _NC is None:
        _NC = build_nc()
    res = run_bass_kernel_spmd(_NC, make_in_maps(inputs), list(range(N_CORES)), trace=trace)
    outputs = np.concatenate([res.results[c]["out"] for c in range(N_CORES)], axis=1)
    new_hidden = np.ascontiguousarray(outputs[STEPS - 1].reshape(1, 1, H))
    return (outputs, new_hidden), res


def kernel(**inputs):
    (outputs, new_hidden), _ = run(inputs)
    return outputs, new_hidden
